# revision 8
# baseline (speedup 1.0000x reference)
"""Trainium2 Bass kernel for nn_BHS_TEST_16724602651186 (gnn_message_passing).

Self-contained: takes FULL inputs (as from reference.setup_inputs()), shards
across 8 NeuronCores internally, returns the FULL (4,4,3) float32 output.

Math (verified against the reference semantics):
  The reference flattens (S,N) into one node axis of S*N rows; edge indices
  are < N, so NNConv message passing only affects s=0 rows.  With
  nn1_b1 == 0 and edge_attr >= 0 (both asserted at runtime), the edge MLP is
  exactly rank-1:  eh[e] = a_e * relu(W1),  so
      agg[n] = (sum_{e->n} a_e * x0[src_e]) @ Wc,
      Wc[f,h] = sum_k relu(W1)_k * W2[f*H+h, k]    (host-folded).
  conv_out[s] = relu(([s==0] u @ Wc) + x[s] @ root_W + conv_b)
  then a 1-layer GRU over s (batch = nodes), then dueling heads.

Sharding: dst-node sharding (1024 nodes/core).  Each core gathers its
in-edges' x0[src] rows (dma_gather), scales by a, and segment-sums via a
host-choreographed staircase tree-fold (nodes degree-sorted per core; row j
holds the j-th in-edge slot of every node with deg > j; bulk strided DVE adds
fold rows pairwise).  GRU runs feature-major (H x nodes).  The wide dueling
head is K-sharded tensor-parallel: each core streams its (65536 x 76) slice
of [val1_W; adv_W]^T (bf16) and accumulates a (4 x 76) partial; partials are
summed on host and the tiny (<40 KFLOP) head tail is applied there.
"""
import os
import numpy as np

import concourse.bacc as bacc
import concourse.bass as bass
import concourse.mybir as mybir
import concourse.tile as tile
from concourse.bass_utils import run_bass_kernel_spmd

F32 = mybir.dt.float32
BF16 = mybir.dt.bfloat16
I16 = mybir.dt.int16
AF = mybir.ActivationFunctionType
ALU = mybir.AluOpType

N, FIN, H, S, E, M = 8192, 16, 64, 4, 131072, 8
NL = N // M            # 1024 dst nodes per core
XW = 64                # gather row width (f32): x0 padded to 256B rows
NT = NL // 128         # node tiles per core (8)
KT = NT * H            # head K-tiles per core (512)
NJ = 76                # head output columns: 64 val1 + 12 adv

LAST_RESULTS = None    # BassKernelResults of the most recent run (for test.py)
_PROGRAM_CACHE = {}


def _roundup(x, m):
    return (x + m - 1) // m * m


# ---------------------------------------------------------------- host plan --
def build_plan(edge, edge_attr):
    src = np.asarray(edge[0], dtype=np.int64)
    dst = np.asarray(edge[1], dtype=np.int64)
    a = np.asarray(edge_attr[:, 0], dtype=np.float32)

    cores, degs = [], np.zeros((M, NL), dtype=np.int64)
    for c in range(M):
        lo = c * NL
        mask = (dst >= lo) & (dst < lo + NL)
        src_c, a_c, dstl = src[mask], a[mask], dst[mask] - lo
        deg = np.bincount(dstl, minlength=NL)
        degs[c] = deg
        cores.append((src_c, a_c, dstl))

    D = max(int(degs.max()), 1)
    sorted_degs = -np.sort(-degs, axis=1)
    m = np.zeros(D, dtype=np.int64)
    m[0] = NL
    for j in range(1, D):
        m[j] = int((sorted_degs > j).sum(axis=1).max())
    P = np.array([_roundup(int(v), 128) for v in m], dtype=np.int64)
    O = np.zeros(D + 1, dtype=np.int64)
    O[1:] = np.cumsum(P)
    T = int(_roundup(O[D], 128))

    folds = []
    cur = D
    while cur > 1:
        half = (cur + 1) // 2
        for j in range(half, cur):
            folds.append((int(O[j - half] // 128), int(O[j] // 128),
                          int(P[j] // 128)))
        cur = half

    idxs = np.zeros((M, T), dtype=np.int16)
    avals = np.zeros((M, T), dtype=np.float32)
    perms = np.zeros((M, NL), dtype=np.int64)
    for c in range(M):
        src_c, a_c, dstl = cores[c]
        order = np.argsort(-degs[c], kind="stable")
        perms[c] = order
        rank_of = np.empty(NL, dtype=np.int64)
        rank_of[order] = np.arange(NL)
        sort_by_dst = np.argsort(dstl, kind="stable")
        dst_sorted = dstl[sort_by_dst]
        starts = np.searchsorted(dst_sorted, np.arange(NL))
        occ = np.arange(len(dstl)) - starts[dst_sorted]
        pos = O[occ] + rank_of[dst_sorted]
        idxs[c, pos] = src_c[sort_by_dst].astype(np.int16)
        avals[c, pos] = a_c[sort_by_dst]
    return dict(T=T, folds=tuple(folds), idxs=idxs, avals=avals, perms=perms)


def _wrap_idxs(lin):
    t = len(lin)
    w = lin.reshape(t // 16, 16).T.astype(np.int16)
    return np.ascontiguousarray(np.tile(w, (8, 1)))


# ------------------------------------------------------------- bass program --
def build_program(T, folds):
    C = T // 128
    nc = bacc.Bacc("TRN2", target_bir_lowering=False, debug=False,
                   num_devices=M, num_swdge_queues=1)
    d = {}
    def din(name, shape, dt=F32):
        d[name] = nc.dram_tensor(name, list(shape), dt, kind="ExternalInput").ap()
    din("x0pad", (N, XW))
    din("gidx", (128, T // 16), I16)
    din("gav", (128, C))
    din("xTloc", (FIN, S * NL))
    din("h0T", (H, NL))
    din("wc", (FIN, H))
    din("rootw", (FIN, H))
    din("convb", (H, 1))
    din("wih", (H, 3 * H))
    din("whh", (H, 3 * H))
    din("grub", (H, 4))
    din("ident", (128, 128))
    din("wheads", (128, KT * NJ), BF16)
    out_d = nc.dram_tensor("partial", [S, NJ], F32, kind="ExternalOutput").ap()

    with tile.TileContext(nc) as tc:
        with (
            tc.tile_pool(name="const", bufs=1) as cpool,
            tc.tile_pool(name="big", bufs=1) as big,
            tc.tile_pool(name="work", bufs=1) as work,
            tc.tile_pool(name="ps_tr", bufs=2, space="PSUM") as ps_tr,
            tc.tile_pool(name="ps_g", bufs=1, space="PSUM") as ps_g,
            tc.tile_pool(name="ps_hd", bufs=1, space="PSUM") as ps_hd,
        ):
            # ---- constant / param loads (HWDGE) ----
            def load(name, shape, dt=F32, pool=cpool):
                t = pool.tile(list(shape), dt, tag=name)
                nc.sync.dma_start(t[:], d[name])
                return t
            ident = load("ident", (128, 128))
            wc = load("wc", (FIN, H))
            rootw = load("rootw", (FIN, H))
            convb = load("convb", (H, 1))
            wih = load("wih", (H, 3 * H))
            whh = load("whh", (H, 3 * H))
            grub = load("grub", (H, 4))
            xTloc = load("xTloc", (FIN, S * NL))
            h0T = load("h0T", (H, NL))
            gidx = load("gidx", (128, T // 16), I16)
            gav = load("gav", (128, C))

            # ---- head weights: 4 chunked DMAs, scheduled early, used late ----
            wsb = big.tile([128, KT, NJ], BF16, tag="wsb")
            wh_flat = wsb[:].rearrange("p k j -> p (k j)")
            for i in range(4):
                sl = slice(i * (KT // 4) * NJ, (i + 1) * (KT // 4) * NJ)
                nc.sync.dma_start(wh_flat[:, sl], d["wheads"][:, sl])

            # ---- gather + scale + staircase fold (segment sum) ----
            # split the gather: with single_packet one engine packet holds at
            # most 64 descriptors -> 1024 idxs per dma_gather instruction
            GCH = 1024
            with tc.tile_pool(name="gat", bufs=1) as gat:
                V64 = gat.tile([128, C, XW], F32, tag="V64")
                for gi, start in enumerate(range(0, T, GCH)):
                    cnt = min(GCH, T - start)
                    nc.gpsimd.dma_gather(
                        V64[:, start // 128:(start + cnt) // 128, :],
                        d["x0pad"],
                        gidx[:, start // 16:(start + cnt) // 16],
                        cnt, cnt, XW)
                V = work.tile([128, C, FIN], F32, tag="V")
                nc.vector.tensor_tensor(
                    V[:], V64[:, :, :FIN],
                    gav[:].unsqueeze(-1).broadcast_to([128, C, FIN]),
                    ALU.mult)
            for dc, sc, nch in folds:
                nc.vector.tensor_tensor(
                    V[:, dc:dc + nch, :], V[:, dc:dc + nch, :],
                    V[:, sc:sc + nch, :], ALU.add)

            # ---- transpose u to (16 x NL) ----
            ut = work.tile([FIN, NL], F32, tag="ut")
            for t in range(NT):
                pt = ps_tr.tile([FIN, 128], F32, tag="ptr")
                nc.tensor.transpose(pt[:], V[:, t, :], ident[:])
                nc.scalar.copy(ut[:, t * 128:(t + 1) * 128], pt[:])

            # ---- phase 1: conv_out (feature-major), all s ----
            xts = work.tile([H, S, NL], F32, tag="xts")
            for s in range(S):
                for ch in range(2):
                    sl = slice(ch * 512, (ch + 1) * 512)
                    p1 = ps_g.tile([H, 512], F32, tag="pr")
                    nc.tensor.matmul(p1[:], rootw[:],
                                     xTloc[:, s * NL:(s + 1) * NL][:, sl],
                                     start=True, stop=(s != 0))
                    if s == 0:
                        nc.tensor.matmul(p1[:], wc[:], ut[:, sl],
                                         start=False, stop=True)
                    nc.scalar.activation(xts[:, s, sl], p1[:], AF.Relu,
                                         bias=convb[:])

            # ---- GRU (feature-major), h in SBUF, ys -> ysbf (node-major) ----
            h = work.tile([H, NL], F32, tag="h")
            nc.vector.tensor_copy(h[:], h0T[:])
            ysbf = work.tile([128, NT, S, H], BF16, tag="ysbf")
            for s in range(S):
                for ch in range(2):
                    sl = slice(ch * 512, (ch + 1) * 512)
                    pr = ps_g.tile([H, 512], F32, tag="pr")
                    pz = ps_g.tile([H, 512], F32, tag="pz")
                    pi = ps_g.tile([H, 512], F32, tag="pi")
                    ph = ps_g.tile([H, 512], F32, tag="ph")
                    xt_sl = xts[:, s, sl]
                    nc.tensor.matmul(pr[:], wih[:, 0:H], xt_sl, start=True, stop=False)
                    nc.tensor.matmul(pr[:], whh[:, 0:H], h[:, sl], start=False, stop=True)
                    nc.tensor.matmul(pz[:], wih[:, H:2 * H], xt_sl, start=True, stop=False)
                    nc.tensor.matmul(pz[:], whh[:, H:2 * H], h[:, sl], start=False, stop=True)
                    nc.tensor.matmul(pi[:], wih[:, 2 * H:3 * H], xt_sl, start=True, stop=True)
                    nc.tensor.matmul(ph[:], whh[:, 2 * H:3 * H], h[:, sl], start=True, stop=True)
                    rt = work.tile([H, 512], F32, tag="rt")
                    zt = work.tile([H, 512], F32, tag="zt")
                    it = work.tile([H, 512], F32, tag="it")
                    ht = work.tile([H, 512], F32, tag="ht")
                    nc.scalar.activation(rt[:], pr[:], AF.Sigmoid, bias=grub[:, 0:1])
                    nc.scalar.activation(zt[:], pz[:], AF.Sigmoid, bias=grub[:, 1:2])
                    nc.scalar.activation(it[:], pi[:], AF.Identity, bias=grub[:, 2:3])
                    nc.scalar.activation(ht[:], ph[:], AF.Identity, bias=grub[:, 3:4])
                    ng = work.tile([H, 512], F32, tag="ng")
                    nc.vector.tensor_mul(ht[:], rt[:], ht[:])     # r * (hn+b)
                    nc.vector.tensor_add(ht[:], ht[:], it[:])
                    nc.scalar.activation(ng[:], ht[:], AF.Tanh)
                    dt_ = work.tile([H, 512], F32, tag="dt_")
                    nc.vector.tensor_sub(dt_[:], h[:, sl], ng[:])  # h - ng
                    nc.vector.tensor_mul(dt_[:], zt[:], dt_[:])
                    nc.vector.tensor_add(h[:, sl], ng[:], dt_[:])  # new h
                for t in range(NT):
                    py = ps_tr.tile([128, H], F32, tag="ptr")
                    nc.tensor.transpose(py[:], h[:, t * 128:(t + 1) * 128],
                                        ident[:H, :H])
                    nc.scalar.copy(ysbf[:, t, s, :], py[:])

            # ---- dueling head partials: accumulate over 512 K-tiles ----
            php = ps_hd.tile([S, NJ], F32, tag="php")
            for k in range(KT):
                t, hh = k // H, k % H
                nc.tensor.matmul(php[:], ysbf[:, t, :, hh], wsb[:, k, :],
                                 start=(k == 0), stop=(k == KT - 1))
            psb = work.tile([S, NJ], F32, tag="psb")
            nc.scalar.copy(psb[:], php[:])
            nc.sync.dma_start(out_d, psb[:])

    nc.compile()
    return nc


# ------------------------------------------------------------------ kernel --
def kernel(**inputs):
    global LAST_RESULTS
    inp = {k: np.asarray(v) for k, v in inputs.items()}

    # --- verify the algebraic collapse assumptions on the actual data ---
    a = inp["edge_attr"].astype(np.float32)
    W1 = inp["nn1_W1"].astype(np.float32)
    eh_ref = np.maximum(a @ W1.T + inp["nn1_b1"][None, :].astype(np.float32), 0.0)
    c1 = np.maximum(W1[:, 0], 0.0)
    if not (np.array_equal(eh_ref, a * c1[None, :])
            and not inp["nn1_b2"].any()):
        raise NotImplementedError(
            "edge-MLP rank-1 collapse does not hold for these inputs")
    Wc = (inp["nn1_W2"].astype(np.float32).reshape(FIN, H, 64)
          * c1[None, None, :]).sum(-1)

    plan = build_plan(inp["edge"], inp["edge_attr"])
    T, folds = plan["T"], plan["folds"]

    key = (T, folds)
    if key not in _PROGRAM_CACHE:
        _PROGRAM_CACHE[key] = build_program(T, folds)
    nc = _PROGRAM_CACHE[key]

    x0 = inp["x"][0].astype(np.float32)                       # (N, 16)
    x0pad = np.zeros((N, XW), dtype=np.float32)
    x0pad[:, :FIN] = x0
    x_all = np.transpose(inp["x"], (1, 0, 2)).reshape(N, S * FIN).astype(np.float32)
    Wcat = np.concatenate([inp["val1_W"], inp["adv_W"]], axis=0).astype(np.float32)

    wih = np.ascontiguousarray(inp["gru_Wih"].astype(np.float32)
                               .reshape(3, H, H).transpose(2, 0, 1)
                               .reshape(H, 3 * H))
    whh = np.ascontiguousarray(inp["gru_Whh"].astype(np.float32)
                               .reshape(3, H, H).transpose(2, 0, 1)
                               .reshape(H, 3 * H))
    bsum = (inp["gru_bih"] + inp["gru_bhh"]).astype(np.float32)
    grub = np.stack([bsum[:H], bsum[H:2 * H],
                     inp["gru_bih"][2 * H:].astype(np.float32),
                     inp["gru_bhh"][2 * H:].astype(np.float32)], axis=1)

    ident = np.eye(128, dtype=np.float32)
    C = T // 128
    in_maps = []
    for c in range(M):
        nodes = c * NL + plan["perms"][c]
        xT = x_all[nodes].reshape(NL, S, FIN).transpose(2, 1, 0)  # (16, S, NL)
        cols = (nodes[:, None] * H + np.arange(H)).ravel()
        import ml_dtypes
        Wsh = Wcat[:, cols].reshape(NJ, NT, 128, H)
        wheads = np.transpose(Wsh, (2, 1, 3, 0)).reshape(128, KT * NJ) \
            .astype(ml_dtypes.bfloat16)
        in_maps.append({
            "x0pad": x0pad,
            "gidx": _wrap_idxs(plan["idxs"][c]),
            "gav": np.ascontiguousarray(
                plan["avals"][c].reshape(C, 128).T),
            "xTloc": np.ascontiguousarray(xT.reshape(FIN, S * NL)),
            "h0T": np.ascontiguousarray(
                inp["h0"][0][nodes].T.astype(np.float32)),
            "wc": Wc,
            "rootw": inp["root_W"].astype(np.float32),
            "convb": inp["conv_b"].astype(np.float32).reshape(H, 1),
            "wih": wih,
            "whh": whh,
            "grub": np.ascontiguousarray(grub),
            "ident": ident,
            "wheads": np.ascontiguousarray(wheads),
        })

    res = run_bass_kernel_spmd(nc, in_maps, core_ids=list(range(M)))
    LAST_RESULTS = res

    partials = np.stack([r["partial"].astype(np.float32) for r in res.results])
    tot = partials.sum(axis=0)
    # tiny head tail (fp32, <40 KFLOP) — part of unsharding/assembly
    v1 = np.maximum(tot[:, :64] + inp["val1_b"].astype(np.float32), 0.0)
    adv = np.maximum(tot[:, 64:] + inp["adv_b"].astype(np.float32), 0.0)
    v2 = np.maximum(v1 @ inp["val2_W"].T.astype(np.float32)
                    + inp["val2_b"].astype(np.float32), 0.0)
    v3 = v2 @ inp["val3_W"].T.astype(np.float32) + inp["val3_b"].astype(np.float32)
    adv = adv.reshape(S, 4, 3)
    out = v3[:, :, None] + adv - adv.mean(-1, keepdims=True)
    return out.astype(np.float32)
